# revision 1
# baseline (speedup 1.0000x reference)
"""Trainium2 Bass kernel for a GPT-style transformer block (B=2, T=2048,
C=1024, 16 heads, MLP 4x), sharded across 8 NeuronCores.

Sharding: rows (batch*token) are data-parallel. Core c owns batch c//4 and
the paired 256-row q-subchunks {j, 7-j} (j = c%4) so causal-attention work
is near-identical on every core -- required because SPMD runs one program on
all cores. The attention loop is a fixed 24 kv-tile iterations (8 for the
low slot, 16 for the high slot); per-core causal structure lives in DATA: a
bf16 multiplicative mask table (ones / zeros / triangular) applied to the
exp'd scores before the p@v accumulation.

Host precomputes LN1 (inputs-only), folds LN scale/shift and the 1/sqrt(D)
into the matmul weights, and pre-transposes everything so the device never
transposes. Device keeps activations as [channel, token]; residual stream
and softmax normalization stay fp32; matmuls are bf16 with fp32 PSUM.
"""
import numpy as np
import ml_dtypes

import concourse.bass as bass
import concourse.mybir as mybir
import concourse.tile as tile
import concourse.bacc as bacc
from concourse.bass_utils import run_bass_kernel_spmd

BF = ml_dtypes.bfloat16
P = 128
B, T, C, H, D, F = 2, 2048, 1024, 16, 64, 4096
NCT = C // P          # 8   c-tiles
NFT = F // P          # 32  f-tiles
NKT = T // P          # 16  kv tiles per batch
SUB = 256             # q subchunk rows
EPS = 1e-5
f32 = mybir.dt.float32
bf16 = mybir.dt.bfloat16
AF = mybir.ActivationFunctionType

_CACHED_NC = None


def _build_nc():
    nc = bacc.Bacc("TRN2", target_bir_lowering=False, debug=False)
    d = {}
    for name, shape, dt in [
        ("gTq", [C, 512], bf16), ("xbT", [C, 512], f32),
        ("WqT", [C, C], bf16), ("WkT", [C, C], bf16), ("WvT", [C, C], bf16),
        ("WpT", [C, C], bf16), ("WupT", [C, F], bf16), ("WdownT", [F, C], bf16),
        ("bq", [P, 8], f32), ("bk", [P, 8], f32), ("bup", [P, 32], f32),
        ("bdown", [P, 8], f32), ("brep", [P, C], bf16),
        ("maskt", [P, 16, 1024], bf16),
    ]:
        d[name] = nc.dram_tensor(name, shape, dt, kind="ExternalInput").ap()
    d["OUT"] = nc.dram_tensor("OUT", [C, 512], f32, kind="ExternalOutput").ap()

    with tile.TileContext(nc) as tc:
        _emit(nc, tc, d)
    nc.compile()
    return nc


def _emit(nc, tc, d):
    from contextlib import ExitStack

    with ExitStack() as ctx:
        # ---- long-lived pools (creation order = SBUF stack order) ----
        cpool = ctx.enter_context(tc.tile_pool(name="cpool", bufs=1))
        bpool_cm = tc.tile_pool(name="bpool", bufs=1)
        bpool = bpool_cm.__enter__()   # closed explicitly after attention

        attnT = cpool.tile([P, 8, 512], bf16, name="attnT")
        xbT = cpool.tile([P, 8, 512], f32, name="xbT")
        bq = cpool.tile([P, 8], f32, name="bq")
        bk = cpool.tile([P, 8], f32, name="bk")
        bup = cpool.tile([P, 32], f32, name="bup")
        bdown = cpool.tile([P, 8], f32, name="bdown")
        brep = cpool.tile([P, C], bf16, name="brep")
        epsT = cpool.tile([P, 1], f32, name="epsT")
        onesb = cpool.tile([P, P], bf16, name="onesb")
        nc.vector.memset(epsT[:], EPS)
        nc.vector.memset(onesb[:], 1.0)
        for t, key in [(bq, "bq"), (bk, "bk"), (bup, "bup"), (bdown, "bdown"),
                       (brep, "brep")]:
            nc.sync.dma_start(t[:], d[key])


        kT = bpool.tile([P, 8, T], bf16, name="kT")
        v_aug = bpool.tile([P, H, NKT * 65], bf16, name="v_aug")
        qT = bpool.tile([P, 8, 512], bf16, name="qT")
        v4 = v_aug[:].rearrange("p h (k e) -> p h k e", e=65)
        nc.vector.memset(v4[:, :, :, 64:65], 1.0)

        # ================= QKV projections =================
        # q/k/v are computed only for this core's own 512 rows (columns of
        # gTq); k/v are then AllGathered across the 4-core batch group and
        # re-laid-out into kT [o, t_global] and v_aug (per-head interleave
        # with a ones column).  Global kv tile kt lives in subchunk s=kt//2,
        # owned by core j(s) = s if s<4 else 7-s, at that owner's local
        # column slot (0 if s<4 else 1)*256 + (kt%2)*128.
        with tc.tile_pool(name="gpool", bufs=1) as gpool, \
             tc.tile_pool(name="wpool", bufs=2) as wpool, \
             tc.tile_pool(name="kvop", bufs=1) as kvop, \
             tc.tile_pool(name="agdr", bufs=1, space="DRAM") as agdr, \
             tc.tile_pool(name="qkps", bufs=3, space="PSUM") as qkps:
            gTq = gpool.tile([P, NCT, 512], bf16, name="gTq")
            nc.sync.dma_start(gTq[:], d["gTq"].rearrange("(ct p) t -> p ct t", p=P))

            # ---- qT ----
            wq = wpool.tile([P, NCT, C], bf16, name="wq", tag="w")
            for _ct in range(NCT):
                nc.sync.dma_start(wq[:, _ct, :],
                                  d["WqT"].rearrange("(ct p) o -> ct p o", p=P)[_ct])
            for ot in range(8):
                pq = qkps.tile([P, 512], f32, name="pq", tag="qk")
                for ct in range(NCT):
                    nc.tensor.matmul(pq[:], wq[:, ct, ot * P:(ot + 1) * P],
                                     gTq[:, ct, :],
                                     start=(ct == 0), stop=(ct == NCT - 1))
                nc.scalar.add(qT[:, ot, :], pq[:], bq[:, ot:ot + 1])

            # ---- k for own rows -> kown DRAM [1024, 512] ----
            wk = wpool.tile([P, NCT, C], bf16, name="wk", tag="w")
            for _ct in range(NCT):
                nc.sync.dma_start(wk[:, _ct, :],
                                  d["WkT"].rearrange("(ct p) o -> ct p o", p=P)[_ct])
            kown = kvop.tile([P, 8, 512], bf16, name="kown")
            # one buffer for both k (rows 0:1024) and v (rows 1024:1536,
            # byte-reinterpreted [512, 1024]) so a single AllGather moves both
            kvown_d = agdr.tile([2048, 512], bf16, name="kvown_d")
            kown_d = kvown_d[0:C, :]
            vown_d = kvown_d[C:2048, :].rearrange("(t two) o -> t (two o)", two=2)
            for ot in range(8):
                pk = qkps.tile([P, 512], f32, name="pk", tag="qk")
                for ct in range(NCT):
                    nc.tensor.matmul(pk[:], wk[:, ct, ot * P:(ot + 1) * P],
                                     gTq[:, ct, :],
                                     start=(ct == 0), stop=(ct == NCT - 1))
                nc.scalar.add(kown[:, ot, :], pk[:], bk[:, ot:ot + 1])
                nc.gpsimd.dma_start(
                    kown_d[:].rearrange("(ot p) t -> ot p t", p=P)[ot],
                    kown[:, ot, :])

            # ---- v for own rows (rows layout + bias) -> vown DRAM [512, C] ----
            wv = wpool.tile([P, NCT, C], bf16, name="wv", tag="w")
            for _ct in range(NCT):
                nc.sync.dma_start(wv[:, _ct, :],
                                  d["WvT"].rearrange("(ct p) o -> ct p o", p=P)[_ct])
            vown = kvop.tile([P, 4, C], bf16, name="vown")
            for tt in range(4):
                for oc in range(2):
                    pv = qkps.tile([P, 512], f32, name="pv", tag="qk")
                    for ct in range(NCT):
                        nc.tensor.matmul(
                            pv[:], gTq[:, ct, tt * P:(tt + 1) * P],
                            wv[:, ct, oc * 512:(oc + 1) * 512],
                            start=(ct == 0), stop=(ct == NCT - 1))
                    nc.vector.tensor_add(vown[:, tt, oc * 512:(oc + 1) * 512],
                                         pv[:], brep[:, oc * 512:(oc + 1) * 512])
                nc.gpsimd.dma_start(
                    vown_d[:].rearrange("(tt p) o -> tt p o", p=P)[tt],
                    vown[:, tt, :])

            # ---- AllGather k+v across the 4-core batch group ----
            agkv = agdr.tile([4 * 2048, 512], bf16, name="agkv")
            rg = [[0, 1, 2, 3], [4, 5, 6, 7]]
            nc.gpsimd.collective_compute(
                "AllGather", mybir.AluOpType.bypass,
                ins=[kvown_d.opt()], outs=[agkv.opt()], replica_groups=rg)

            # ---- rebuild kT / v_aug from the gathered tensors ----
            for kt in range(NKT):
                sub = kt // 2
                j = sub if sub < 4 else 7 - sub
                loc = (0 if sub < 4 else 256) + (kt % 2) * P
                src = agkv[j * 2048:j * 2048 + C, loc:loc + P]
                nc.gpsimd.dma_start(
                    kT[:, :, kt * P:(kt + 1) * P],
                    src.rearrange("(ot p) t -> p ot t", p=P))
                vagv = agkv[j * 2048 + C:(j + 1) * 2048, :].rearrange(
                    "(t two) o -> t (two o)", two=2)
                vsrc = vagv[loc:loc + P, :]
                for oc in range(2):
                    nc.gpsimd.dma_start(
                        v4[:, oc * 8:(oc + 1) * 8, kt, 0:64],
                        vsrc[:, oc * 512:(oc + 1) * 512]
                        .rearrange("p (h dd) -> p h dd", dd=64))

        # ================= attention =================
        # fixed iteration space: slot 0 (low subchunk) x kv tiles [0,8),
        # slot 1 (high subchunk) x kv tiles [0,16).  Causality is in the
        # per-core mask data: slot0 all 8 and slot1 kt>=8 get masked
        # (ones / zeros / triangular); slot1 kt<8 is always fully valid.
        with tc.tile_pool(name="mkp", bufs=1) as mkp, \
             tc.tile_pool(name="expp", bufs=3) as expp, \
             tc.tile_pool(name="drp", bufs=4) as drp, \
             tc.tile_pool(name="dramp", bufs=2, space="DRAM") as dramp, \
             tc.tile_pool(name="scps", bufs=2, space="PSUM") as scps, \
             tc.tile_pool(name="avps", bufs=4, space="PSUM") as avps:
            maskt = mkp.tile([P, 16, 1024], bf16, name="maskt")
            for _kt in range(0, 16, 4):
                nc.sync.dma_start(maskt[:, _kt:_kt + 4, :],
                                  d["maskt"][:, _kt:_kt + 4, :])
            iters = [(0, kt) for kt in range(8)] + \
                    [(1, kt) for kt in range(16)]
            for hg in range(4):
                avs = []
                for hI in range(4):
                    av = avps.tile([P, 512], f32, name=f"av{hg}_{hI}", tag="av")
                    nc.vector.memset(av[:], 0.0)
                    avs.append(av)
                for slot, kt in iters:
                    qlo = slot * 256
                    sc = scps.tile([P, 1024], f32, name="sc", tag="sc")
                    for hI in range(4):
                        h = hg * 4 + hI
                        colo = (hI % 2) * 512 + (hI // 2) * 256
                        hb = (h % 2) * 64
                        nc.tensor.matmul(
                            sc[:, colo:colo + 256],
                            kT[hb:hb + 64, h // 2, kt * P:(kt + 1) * P],
                            qT[hb:hb + 64, h // 2, qlo:qlo + 256],
                            start=True, stop=True)
                    ex = expp.tile([P, 1024], bf16, name="ex", tag="ex")
                    nc.scalar.activation(ex[:], sc[:], AF.Exp)
                    if slot == 0 or kt >= 8:
                        nc.vector.tensor_mul(ex[:], ex[:], maskt[:, kt, :])
                    for hI in range(4):
                        h = hg * 4 + hI
                        colo = (hI % 2) * 512 + (hI // 2) * 256
                        nc.tensor.matmul(
                            avs[hI][0:65, qlo:qlo + 256],
                            v_aug[:, h, kt * 65:kt * 65 + 65],
                            ex[:, colo:colo + 256],
                            start=False,
                            stop=(kt == (7 if slot == 0 else 15)),
                            skip_group_check=True)
                # free the av PSUM banks fast: copy raw av (incl the
                # denominator row 64) to SBUF, then normalize from there
                # while the next head group's matmuls proceed.
                avsb = []
                for hI in range(4):
                    a_sb = drp.tile([P, 512], f32, name=f"avsb{hI}",
                                    tag=f"avsb{hI}")
                    nc.scalar.copy(a_sb[0:65, :], avs[hI][0:65, :])
                    avsb.append(a_sb)
                stg = drp.tile([P, 512], f32, name="stg", tag="stg")
                nc.vector.memset(stg[:], 1.0)
                for hI in range(4):
                    nc.vector.tensor_copy(stg[hI * 32:hI * 32 + 1, :],
                                          avsb[hI][64:65, :])
                nc.vector.reciprocal(stg[:], stg[:])
                dend = dramp.tile([4, 512], f32, name="dend", tag="dend")
                for hI in range(4):
                    nc.sync.dma_start(dend[hI:hI + 1, :],
                                      stg[hI * 32:hI * 32 + 1, :])
                for hI in range(4):
                    h = hg * 4 + hI
                    hb = (h % 2) * 64
                    rb = drp.tile([64, 512], f32, name="rb", tag="rb")
                    nc.sync.dma_start(rb[:],
                                      dend[hI:hI + 1, :].to_broadcast((64, 512)))
                    nc.vector.tensor_mul(attnT[hb:hb + 64, h // 2, :],
                                         avsb[hI][0:64, :], rb[:])

        bpool_cm.__exit__(None, None, None)   # free kT / v_aug / qT

        # ================= proj + residual =================
        with tc.tile_pool(name="x1p", bufs=1) as x1p:
            x1T = x1p.tile([P, 8, 512], f32, name="x1T")
            nc.sync.dma_start(xbT[:],
                              d["xbT"].rearrange("(ot p) t -> p ot t", p=P))
            with tc.tile_pool(name="wpp", bufs=1) as wpp, \
                 tc.tile_pool(name="pps", bufs=2, space="PSUM") as pps:
                wp = wpp.tile([P, NCT, C], bf16, name="wp")
                for _ct in range(NCT):
                    nc.sync.dma_start(wp[:, _ct, :],
                                      d["WpT"].rearrange("(ct p) o -> ct p o", p=P)[_ct])
                for ot in range(8):
                    pp = pps.tile([P, 512], f32, name="pp", tag="pp")
                    for ct in range(NCT):
                        nc.tensor.matmul(pp[:], wp[:, ct, ot * P:(ot + 1) * P],
                                         attnT[:, ct, :],
                                         start=(ct == 0), stop=(ct == NCT - 1))
                    nc.vector.tensor_add(x1T[:, ot, :], pp[:], xbT[:, ot, :])

            # ================= LN2 =================
            with tc.tile_pool(name="g2p", bufs=1) as g2p:
                g2 = g2p.tile([P, 8, 512], bf16, name="g2")
                with tc.tile_pool(name="ln2p", bufs=1) as ln2p, \
                     tc.tile_pool(name="lnps", bufs=2, space="PSUM") as lnps:
                    x1b = ln2p.tile([P, 8, 512], bf16, name="x1b")
                    sqb = ln2p.tile([P, 8, 512], bf16, name="sqb")
                    for ot in range(8):
                        nc.vector.tensor_copy(x1b[:, ot, :], x1T[:, ot, :])
                        nc.scalar.activation(sqb[:, ot, :], x1T[:, ot, :], AF.Square)
                    psmu = lnps.tile([P, 512], f32, name="psmu", tag="ln")
                    pssq = lnps.tile([P, 512], f32, name="pssq", tag="ln")
                    for ct in range(NCT):
                        nc.tensor.matmul(psmu[:], onesb[:], x1b[:, ct, :],
                                         start=(ct == 0), stop=(ct == NCT - 1))
                    for ct in range(NCT):
                        nc.tensor.matmul(pssq[:], onesb[:], sqb[:, ct, :],
                                         start=(ct == 0), stop=(ct == NCT - 1))
                    mu = ln2p.tile([P, 512], f32, name="mu")
                    nc.scalar.mul(mu[:], psmu[:], 1.0 / C)
                    e2 = ln2p.tile([P, 512], f32, name="e2")
                    nc.scalar.mul(e2[:], pssq[:], 1.0 / C)
                    musq = ln2p.tile([P, 512], f32, name="musq")
                    nc.scalar.activation(musq[:], mu[:], AF.Square)
                    nc.vector.tensor_sub(e2[:], e2[:], musq[:])
                    std = ln2p.tile([P, 512], f32, name="std")
                    nc.scalar.activation(std[:], e2[:], AF.Sqrt, bias=epsT[:])
                    nc.vector.reciprocal(std[:], std[:])
                    for ct in range(NCT):
                        tmpc = ln2p.tile([P, 512], f32, name="tmpc", tag="tmpc",
                                         bufs=2)
                        nc.vector.tensor_sub(tmpc[:], x1T[:, ct, :], mu[:])
                        nc.vector.tensor_mul(g2[:, ct, :], tmpc[:], std[:])

                # ================= MLP up (gelu) =================
                with tc.tile_pool(name="hp", bufs=1) as hp:
                    hT = hp.tile([P, NFT, 512], bf16, name="hT")
                    with tc.tile_pool(name="wup", bufs=1) as wupp, \
                         tc.tile_pool(name="upps", bufs=2, space="PSUM") as upps:
                        wu = wupp.tile([P, NCT, F], bf16, name="wu")
                        wusrc = d["WupT"].rearrange("(ct p) (oh o) -> ct oh p o",
                                                    p=P, oh=4)
                        for _ct in range(NCT):
                            for _oh in range(4):
                                nc.sync.dma_start(
                                    wu[:, _ct, _oh * 1024:(_oh + 1) * 1024],
                                    wusrc[_ct, _oh])
                        for ot in range(NFT):
                            pu = upps.tile([P, 512], f32, name="pu", tag="pu")
                            for ct in range(NCT):
                                nc.tensor.matmul(
                                    pu[:], wu[:, ct, ot * P:(ot + 1) * P],
                                    g2[:, ct, :],
                                    start=(ct == 0), stop=(ct == NCT - 1))
                            nc.scalar.activation(hT[:, ot, :], pu[:], AF.Gelu,
                                                 bias=bup[:, ot:ot + 1])

                    # ================= MLP down + residual =================
                    with tc.tile_pool(name="wdp", bufs=1) as wdp, \
                         tc.tile_pool(name="outp", bufs=1) as outp, \
                         tc.tile_pool(name="dps", bufs=2, space="PSUM") as dps:
                        wd = wdp.tile([P, NFT, C], bf16, name="wd")
                        wdsrc = d["WdownT"].rearrange("(cf p) o -> cf p o", p=P)
                        for _cf in range(NFT):
                            nc.sync.dma_start(wd[:, _cf, :], wdsrc[_cf])
                        outT = outp.tile([P, 8, 512], f32, name="outT")
                        for ot in range(8):
                            pd = dps.tile([P, 512], f32, name="pd", tag="pd")
                            for cf in range(NFT):
                                nc.tensor.matmul(
                                    pd[:], wd[:, cf, ot * P:(ot + 1) * P],
                                    hT[:, cf, :],
                                    start=(cf == 0), stop=(cf == NFT - 1))
                            td = outp.tile([P, 512], f32, name="td", tag="td",
                                           bufs=2)
                            nc.scalar.add(td[:], pd[:], bdown[:, ot:ot + 1])
                            nc.vector.tensor_add(outT[:, ot, :], td[:],
                                                 x1T[:, ot, :])
                        outdst = d["OUT"].rearrange("(ot p) t -> ot p t", p=P)
                        for ot in range(8):
                            nc.sync.dma_start(outdst[ot], outT[:, ot, :])


def _prep_inputs(x, ln1_w, ln1_b, c_attn_w, c_attn_b, c_proj_w, c_proj_b,
                 ln2_w, ln2_b, up_w, up_b, down_w, down_b):
    """Host-side preprocessing -> list of 8 per-core input dicts."""
    x = np.asarray(x, np.float32)
    f64 = np.float64
    # LN1 on host (pure function of the input)
    mu = x.mean(-1, keepdims=True, dtype=f64)
    var = np.asarray(x, f64).var(-1, keepdims=True)
    g = ((x - mu) / np.sqrt(var + EPS)).astype(np.float32)     # [B, T, C]

    ln1_w = np.asarray(ln1_w, np.float32); ln1_b = np.asarray(ln1_b, np.float32)
    ln2_w = np.asarray(ln2_w, np.float32); ln2_b = np.asarray(ln2_b, np.float32)
    c_attn_w = np.asarray(c_attn_w, np.float32)
    c_attn_b = np.asarray(c_attn_b, np.float32)
    c_proj_w = np.asarray(c_proj_w, np.float32)
    c_proj_b = np.asarray(c_proj_b, np.float32)
    up_w = np.asarray(up_w, np.float32); up_b = np.asarray(up_b, np.float32)
    down_w = np.asarray(down_w, np.float32)
    down_b = np.asarray(down_b, np.float32)

    Wa = c_attn_w * ln1_w[None, :]                  # fold LN1 scale
    ba = c_attn_b + c_attn_w @ ln1_b                # fold LN1 shift
    Wq, Wk, Wv = Wa[:C], Wa[C:2 * C], Wa[2 * C:]
    bqv, bkv, bvv = ba[:C], ba[C:2 * C], ba[2 * C:]
    s = 1.0 / np.sqrt(D)
    Wq = Wq * s; bqv = bqv * s                      # fold attention scale

    Wup = up_w * ln2_w[None, :]
    bupv = up_b + up_w @ ln2_b

    def b2t(v, n):   # per-partition bias layout [128, n]
        return np.ascontiguousarray(v.reshape(n, P).T.astype(np.float32))

    # causal masks for a 128-wide kv tile vs a 256-row q subchunk, by the
    # kv tile's position relative to the subchunk's diagonal
    tk = np.arange(P)[:, None]
    tq = np.arange(SUB)[None, :]
    mA = (tk <= tq).astype(np.float32)        # first diagonal tile
    mB = (tk + P <= tq).astype(np.float32)    # second diagonal tile
    ones_m = np.ones((P, SUB), np.float32)
    zero_m = np.zeros((P, SUB), np.float32)

    def mask_for(rel):
        if rel < 0:
            return ones_m
        if rel == 0:
            return mA
        if rel == 1:
            return mB
        return zero_m

    shared = {
        "WqT": np.ascontiguousarray(Wq.T).astype(BF),
        "WkT": np.ascontiguousarray(Wk.T).astype(BF),
        "WvT": np.ascontiguousarray(Wv.T).astype(BF),
        "WpT": np.ascontiguousarray(c_proj_w.T).astype(BF),
        "WupT": np.ascontiguousarray(Wup.T).astype(BF),
        "WdownT": np.ascontiguousarray(down_w.T).astype(BF),
        "bq": b2t(bqv, 8), "bk": b2t(bkv, 8),
        "bup": b2t(bupv, 32), "bdown": b2t(down_b, 8),
        "brep": np.broadcast_to(bvv.astype(BF), (P, C)).copy(),
    }

    xb = x + c_proj_b[None, None, :]                # fold proj bias into residual
    in_maps, col_ranges = [], []
    for core in range(8):
        b, j = core // 4, core % 4
        subs = [j, 7 - j]
        cols = np.r_[subs[0] * SUB:(subs[0] + 1) * SUB,
                     subs[1] * SUB:(subs[1] + 1) * SUB]
        gTb = g[b].T                                 # [C, T] (view)
        # mask table: rows kt<8 serve slot0 (sub j), kt>=8 serve slot1
        maskt = np.empty((P, 16, 4 * SUB), np.float32)
        for kt in range(8):
            maskt[:, kt, :] = np.tile(mask_for(kt - 2 * subs[0]), (1, 4))
        for kt in range(8, 16):
            maskt[:, kt, :] = np.tile(mask_for(kt - 2 * subs[1]), (1, 4))
        m = dict(shared)
        m["gTq"] = np.ascontiguousarray(gTb[:, cols]).astype(BF)
        m["xbT"] = np.ascontiguousarray(xb[b].T[:, cols]).astype(np.float32)
        m["maskt"] = maskt.astype(BF)
        in_maps.append(m)
        col_ranges.append((b, subs))
    return in_maps, col_ranges


def kernel(**inputs):
    global _CACHED_NC
    if _CACHED_NC is None:
        _CACHED_NC = _build_nc()
    nc = _CACHED_NC
    in_maps, col_ranges = _prep_inputs(**inputs)
    try:
        res = run_bass_kernel_spmd(nc, in_maps, list(range(8)))
    except Exception:
        # one retry: transient NRT device faults are recoverable on re-run
        res = run_bass_kernel_spmd(nc, in_maps, list(range(8)))
    out = np.empty((B, T, C), np.float32)
    for core in range(8):
        o = res.results[core]["OUT"]                # [C, 512]
        b, subs = col_ranges[core]
        out[b, subs[0] * SUB:(subs[0] + 1) * SUB, :] = o[:, :SUB].T
        out[b, subs[1] * SUB:(subs[1] + 1) * SUB, :] = o[:, SUB:].T
    return out



# revision 8
# speedup vs baseline: 1.7631x; 1.7631x over previous
"""Trainium2 Bass kernel for a GPT-style transformer block (B=2, T=2048,
C=1024, 16 heads, MLP 4x), sharded across 8 NeuronCores.

Sharding v2: attention is head-sharded (core 4b+j owns batch b, heads
[4j,4j+4) over ALL 2048 tokens -> exact causal tiling, no kv exchange);
the proj partial output (token-major) is summed+resharded by two chunked
bf16 ReduceScatters (tokens [0,1024) and [1024,2048)), each core receiving
256 tokens per chunk; LN2+MLP run token-sharded on the core's 512 tokens.
The RS output arrives token-major and is flipped back to channel-major by
dma_start_transpose (verified: out[p,s,t] = in[t, 128*s+p]).

Host precomputes LN1, folds LN scale/shift and 1/sqrt(D) into weights,
pre-transposes everything. Residual stays f32 on the output path; matmuls
bf16 with f32 PSUM.
"""
import numpy as np
import ml_dtypes

import concourse.bass as bass
import concourse.mybir as mybir
import concourse.tile as tile
import concourse.bacc as bacc
from concourse.bass_utils import run_bass_kernel_spmd

BF = ml_dtypes.bfloat16
P = 128
B, T, C, H, D, F = 2, 2048, 1024, 16, 64, 4096
NCT = C // P          # 8   c-tiles
NFT = F // P          # 32  f-tiles
NKT = T // P          # 16  kv tiles
EPS = 1e-5
f32 = mybir.dt.float32
bf16 = mybir.dt.bfloat16
AF = mybir.ActivationFunctionType

_CACHED_NC = None


def _build_nc():
    nc = bacc.Bacc("TRN2", target_bir_lowering=False, debug=False)
    d = {}
    for name, shape, dt in [
        ("gT", [C, T], bf16),
        ("WqT", [C, 256], bf16), ("WkT", [C, 256], bf16), ("WvT", [C, 256], bf16),
        ("WpT", [256, C], bf16), ("WupT", [C, F], bf16), ("WdownT", [F, C], bf16),
        ("xbT", [C, 512], bf16),
        ("bq", [P, 2], f32), ("bk", [P, 2], f32), ("brep", [P, 256], bf16),
        ("bup", [P, 32], f32), ("bdown", [P, 8], f32),
        ("maskA", [P, 1024], bf16), ("maskB", [P, 1024], bf16),
    ]:
        d[name] = nc.dram_tensor(name, shape, dt, kind="ExternalInput").ap()
    d["OUT"] = nc.dram_tensor("OUT", [C, 512], f32, kind="ExternalOutput").ap()

    with tile.TileContext(nc) as tc:
        _emit(nc, tc, d)
    nc.compile()
    return nc


def _emit(nc, tc, d):
    from contextlib import ExitStack

    with ExitStack() as ctx:
        # ---------------- long-lived pools ----------------
        cpool = ctx.enter_context(tc.tile_pool(name="cpool", bufs=1))
        wpool = ctx.enter_context(tc.tile_pool(name="wpool", bufs=1))
        dramp = ctx.enter_context(tc.tile_pool(name="dramp", bufs=1, space="DRAM"))

        attnT = cpool.tile([P, 2, T], bf16, name="attnT")      # 8KB/part
        xbT = cpool.tile([P, 8, 512], bf16, name="xbT")        # 8KB
        x1T = cpool.tile([P, 8, 512], bf16, name="x1T")        # 8KB
        maskA = cpool.tile([P, 1024], bf16, name="maskA")      # 2KB
        maskB = cpool.tile([P, 1024], bf16, name="maskB")      # 2KB
        bq = cpool.tile([P, 2], f32, name="bq")
        bk = cpool.tile([P, 2], f32, name="bk")
        brep = cpool.tile([P, 256], bf16, name="brep")
        bup = cpool.tile([P, 32], f32, name="bup")
        bdown = cpool.tile([P, 8], f32, name="bdown")
        epsT = cpool.tile([P, 1], f32, name="epsT")
        onesb = cpool.tile([P, P], bf16, name="onesb")
        nc.vector.memset(epsT[:], EPS)
        nc.vector.memset(onesb[:], 1.0)

        wup = wpool.tile([P, NCT, F], bf16, name="wup")        # 64KB

        rs_in = [dramp.tile([1024, C], bf16, name=f"rs_in{c}") for c in range(2)]
        rs_out = [dramp.tile([256, C], bf16, name=f"rs_out{c}") for c in range(2)]

        for t, key in [(bq, "bq"), (bk, "bk"), (brep, "brep"), (bup, "bup"),
                       (bdown, "bdown"), (maskA, "maskA"), (maskB, "maskB")]:
            nc.sync.dma_start(t[:], d[key])

        # =========== phase 1: QKV + attention + proj ===========
        with tc.tile_pool(name="p1", bufs=1) as p1:
            qT = p1.tile([P, 2, T], bf16, name="qT")           # 8KB
            kT = p1.tile([P, 2, T], bf16, name="kT")           # 8KB
            v_aug = p1.tile([P, 4, NKT * 65], bf16, name="v_aug")   # 8.3KB
            wp = p1.tile([P, 2, C], bf16, name="wp")           # 4KB
            v4 = v_aug[:].rearrange("p h (k e) -> p h k e", e=65)
            nc.vector.memset(v4[:, :, :, 64:65], 1.0)
            nc.sync.dma_start(wp[:],
                              d["WpT"].rearrange("(ct p) o -> p ct o", p=P))
            # weight DMAs for the MLP launched early (prefetch)
            wusrc = d["WupT"].rearrange("(ct p) f -> p ct f", p=P)
            wdsrc = d["WdownT"].rearrange("(cf p) o -> p cf o", p=P)

            # ---- QKV projections ----
            with tc.tile_pool(name="gp", bufs=1) as gp, \
                 tc.tile_pool(name="qkps", bufs=3, space="PSUM") as qkps:
                gT = gp.tile([P, NCT, T], bf16, name="gT")     # 16KB
                wq = gp.tile([P, NCT, 256], bf16, name="wq")
                wk = gp.tile([P, NCT, 256], bf16, name="wk")
                wv = gp.tile([P, NCT, 256], bf16, name="wv")
                gsrc = d["gT"].rearrange("(ct p) t -> p ct t", p=P)
                for tch in range(4):
                    nc.sync.dma_start(gT[:, :, tch * 512:(tch + 1) * 512],
                                      gsrc[:, :, tch * 512:(tch + 1) * 512])
                for w, key in [(wq, "WqT"), (wk, "WkT"), (wv, "WvT")]:
                    nc.sync.dma_start(w[:],
                                      d[key].rearrange("(ct p) o -> p ct o", p=P))
                nc.sync.dma_start(xbT[:],
                                  d["xbT"].rearrange("(ot p) t -> p ot t", p=P))
                nc.sync.dma_start(wup[:], wusrc)

                for tch in range(4):
                    tsl = slice(tch * 512, (tch + 1) * 512)
                    for w, dst, b in [(wk, kT, bk), (wq, qT, bq)]:
                        for ot in range(2):
                            pq = qkps.tile([P, 512], f32, name="pq", tag="qk")
                            for ct in range(NCT):
                                nc.tensor.matmul(
                                    pq[:], w[:, ct, ot * P:(ot + 1) * P],
                                    gT[:, ct, tsl],
                                    start=(ct == 0), stop=(ct == NCT - 1))
                            nc.scalar.add(dst[:, ot, tsl], pq[:], b[:, ot:ot + 1])
                    for tt in range(4 * tch, 4 * tch + 4):
                        pv = qkps.tile([P, 256], f32, name="pv", tag="qk")
                        for ct in range(NCT):
                            nc.tensor.matmul(
                                pv[:], gT[:, ct, tt * P:(tt + 1) * P],
                                wv[:, ct, :],
                                start=(ct == 0), stop=(ct == NCT - 1))
                        nc.vector.tensor_add(
                            v4[:, :, tt, 0:64],
                            pv[:].rearrange("p (h dd) -> p h dd", dd=64),
                            brep[:].rearrange("p (h dd) -> p h dd", dd=64))

            # ---- attention (software-pipelined) + proj chunks ----
            with tc.tile_pool(name="expp", bufs=3) as expp, \
                 tc.tile_pool(name="drp", bufs=2) as drp, \
                 tc.tile_pool(name="rscp", bufs=2) as rscp, \
                 tc.tile_pool(name="scps", bufs=2, space="PSUM") as scps, \
                 tc.tile_pool(name="avps", bufs=2, space="PSUM") as avps:

                def emit_av(av, ex, k, last):
                    for h in range(4):
                        nc.tensor.matmul(
                            av[0:65, h * 256:(h + 1) * 256],
                            v_aug[:, h, k * 65:k * 65 + 65],
                            ex[:, h * 256:(h + 1) * 256],
                            start=(k == 0 and h % 2 == 0), stop=last,
                            skip_group_check=True)

                def emit_proj(c, tt):
                    # proj partial, token-major: tokens [c*1024+tt*128, +128)
                    pp = scps.tile([P, 1024], f32, name=f"pp{c}_{tt}", tag="sc")
                    for oc in range(2):
                        for ct in range(2):
                            nc.tensor.matmul(
                                pp[:, oc * 512:(oc + 1) * 512],
                                attnT[:, ct, c * 1024 + tt * P:
                                      c * 1024 + (tt + 1) * P],
                                wp[:, ct, oc * 512:(oc + 1) * 512],
                                start=(ct == 0), stop=(ct == 1))
                    rsct = rscp.tile([P, C], bf16, name=f"rsc{c}_{tt}",
                                     tag="rsc")
                    nc.vector.tensor_copy(rsct[:], pp[:])
                    nc.sync.dma_start(
                        rs_in[c][tt * P:(tt + 1) * P, :]
                        .rearrange("p o -> p o"), rsct[:])

                def send_rs(c):
                    nc.gpsimd.collective_compute(
                        "ReduceScatter", mybir.AluOpType.add,
                        ins=[rs_in[c].opt()], outs=[rs_out[c].opt()],
                        replica_groups=[[0, 1, 2, 3], [4, 5, 6, 7]])

                for qg in range(8):
                    K = 2 * qg + 2
                    qsl = slice(qg * 256, (qg + 1) * 256)
                    av = avps.tile([P, 1024], f32, name=f"av{qg}", tag="av")
                    prev = None
                    for k in range(K):
                        sc = scps.tile([P, 1024], f32, name=f"sc{qg}_{k}",
                                       tag="sc")
                        for h in range(4):
                            hb = (h % 2) * 64
                            nc.tensor.matmul(
                                sc[:, h * 256:(h + 1) * 256],
                                kT[hb:hb + 64, h // 2, k * P:(k + 1) * P],
                                qT[hb:hb + 64, h // 2, qsl],
                                start=True, stop=True)
                        ex = expp.tile([P, 1024], bf16, name=f"ex{qg}_{k}",
                                       tag="ex")
                        nc.scalar.activation(ex[:], sc[:], AF.Exp)
                        if k == 2 * qg:
                            nc.vector.tensor_mul(ex[:], ex[:], maskA[:])
                        elif k == 2 * qg + 1:
                            nc.vector.tensor_mul(ex[:], ex[:], maskB[:])
                        if prev is not None:
                            emit_av(av, prev[0], prev[1], False)
                        prev = (ex, k)
                    emit_av(av, prev[0], prev[1], True)
                    # epilogue: normalize via reciprocal + K=1 broadcast matmul
                    avsb = drp.tile([P, 1024], bf16, name=f"avsb{qg}",
                                    tag="avsb")
                    nc.vector.tensor_copy(avsb[0:65, :], av[0:65, :])
                    denr = drp.tile([1, 1024], bf16, name=f"denr{qg}",
                                    tag="denr")
                    with nc.allow_low_precision(reason="softmax denom bf16"):
                        nc.vector.reciprocal(denr[:], avsb[64:65, :])
                    for h in range(4):
                        nc.tensor.matmul(
                            av[64:128, h * 256:(h + 1) * 256],
                            onesb[0:1, 0:64], denr[0:1, h * 256:(h + 1) * 256],
                            start=True, stop=True, skip_group_check=True)
                    for h in range(4):
                        hb = (h % 2) * 64
                        nc.vector.tensor_mul(
                            attnT[hb:hb + 64, h // 2, qsl],
                            avsb[0:64, h * 256:(h + 1) * 256],
                            av[64:128, h * 256:(h + 1) * 256])
                    # interleave proj: chunk0 spread over qg4/qg5, chunk1 after qg7
                    if qg == 4:
                        for tt in range(4):
                            emit_proj(0, tt)
                    elif qg == 5:
                        for tt in range(4, 8):
                            emit_proj(0, tt)
                        send_rs(0)
                    elif qg == 7:
                        for tt in range(8):
                            emit_proj(1, tt)
                        send_rs(1)

        # =========== phase 2: LN2 + MLP, per 256-token chunk ===========
        with tc.tile_pool(name="mlp", bufs=1) as mlp, \
             tc.tile_pool(name="sml", bufs=1) as sml, \
             tc.tile_pool(name="outp", bufs=1) as outp, \
             tc.tile_pool(name="mps", bufs=2, space="PSUM") as mps, \
             tc.tile_pool(name="ups", bufs=2, space="PSUM") as ups, \
             tc.tile_pool(name="dps", bufs=2, space="PSUM") as dps:
            outdst = d["OUT"].rearrange("(ot p) t -> p ot t", p=P)
            wdown = mlp.tile([P, NFT, C], bf16, name="wdown")   # 64KB
            nc.sync.dma_start(wdown[:], wdsrc)
            for c in range(2):
                csl = slice(c * 256, (c + 1) * 256)
                x1g = sml.tile([P, 8, 256], bf16, name=f"x1g{c}", tag="x1g")
                nc.sync.dma_start_transpose(x1g[:], rs_out[c][:])
                with nc.allow_low_precision(reason="residual bf16"):
                    nc.vector.tensor_add(x1T[:, :, csl], x1g[:],
                                         xbT[:, :, csl])
                # LN2 stats via ones-matmul (sum over channel partitions)
                sqb = sml.tile([P, 8, 256], bf16, name=f"sqb{c}", tag="sqb")
                nc.scalar.activation(sqb[:], x1T[:, :, csl], AF.Square)
                psmu = mps.tile([P, 256], f32, name=f"psmu{c}", tag="mu")
                pssq = mps.tile([P, 256], f32, name=f"pssq{c}", tag="mu")
                for ct in range(NCT):
                    nc.tensor.matmul(psmu[:], onesb[:], x1T[:, ct, csl],
                                     start=(ct == 0), stop=(ct == NCT - 1))
                for ct in range(NCT):
                    nc.tensor.matmul(pssq[:], onesb[:], sqb[:, ct, :],
                                     start=(ct == 0), stop=(ct == NCT - 1))
                mu = sml.tile([P, 256], f32, name=f"mu{c}", tag="mu2")
                nc.scalar.mul(mu[:], psmu[:], 1.0 / C)
                e2 = sml.tile([P, 256], f32, name=f"e2{c}", tag="e2")
                nc.scalar.mul(e2[:], pssq[:], 1.0 / C)
                musq = sml.tile([P, 256], f32, name=f"musq{c}", tag="musq")
                nc.scalar.activation(musq[:], mu[:], AF.Square)
                nc.vector.tensor_sub(e2[:], e2[:], musq[:])
                std = sml.tile([P, 256], f32, name=f"std{c}", tag="std")
                nc.scalar.activation(std[:], e2[:], AF.Sqrt, bias=epsT[:])
                nc.vector.reciprocal(std[:], std[:])
                g2 = sml.tile([P, 8, 256], bf16, name=f"g2{c}", tag="g2")
                with nc.allow_low_precision(reason="ln2 bf16"):
                    for ct in range(NCT):
                        nc.vector.tensor_sub(sqb[:, ct, :], x1T[:, ct, csl],
                                             mu[:])
                        nc.vector.tensor_mul(g2[:, ct, :], sqb[:, ct, :],
                                             std[:])
                # ---- up + gelu ----
                hT = mlp.tile([P, NFT, 256], bf16, name=f"hT{c}", tag="hT")
                for fg in range(NFT // 2):
                    pu = ups.tile([P, 512], f32, name=f"pu{c}_{fg}", tag="pu")
                    for sub in range(2):
                        ft = fg * 2 + sub
                        for ct in range(NCT):
                            nc.tensor.matmul(
                                pu[:, sub * 256:(sub + 1) * 256],
                                wup[:, ct, ft * P:(ft + 1) * P],
                                g2[:, ct, :],
                                start=(ct == 0 and sub == 0),
                                stop=(ct == NCT - 1),
                                skip_group_check=True)
                    for sub in range(2):
                        ft = fg * 2 + sub
                        nc.scalar.activation(
                            hT[:, ft, :], pu[:, sub * 256:(sub + 1) * 256],
                            AF.Gelu, bias=bup[:, ft:ft + 1])
                # ---- down + bias + residual ----
                outO = outp.tile([P, 8, 256], f32, name=f"outO{c}", tag="outO")
                for ot in range(8):
                    pd = dps.tile([P, 256], f32, name=f"pd{c}_{ot}", tag="pd")
                    for cf in range(NFT):
                        nc.tensor.matmul(pd[:], wdown[:, cf, ot * P:(ot + 1) * P],
                                         hT[:, cf, :],
                                         start=(cf == 0), stop=(cf == NFT - 1))
                    td = outp.tile([P, 256], f32, name=f"td{c}_{ot}", tag="td",
                                   bufs=2)
                    nc.scalar.add(td[:], pd[:], bdown[:, ot:ot + 1])
                    nc.vector.tensor_add(outO[:, ot, :], td[:], x1T[:, ot, csl])
                nc.sync.dma_start(outdst[:, :, csl], outO[:])


def _prep_inputs(x, ln1_w, ln1_b, c_attn_w, c_attn_b, c_proj_w, c_proj_b,
                 ln2_w, ln2_b, up_w, up_b, down_w, down_b):
    """Host-side preprocessing -> list of 8 per-core input dicts."""
    x = np.asarray(x, np.float32)
    f64 = np.float64
    mu = x.mean(-1, keepdims=True, dtype=f64)
    var = np.asarray(x, f64).var(-1, keepdims=True)
    g = ((x - mu) / np.sqrt(var + EPS)).astype(np.float32)     # [B, T, C]

    ln1_w = np.asarray(ln1_w, np.float32); ln1_b = np.asarray(ln1_b, np.float32)
    ln2_w = np.asarray(ln2_w, np.float32); ln2_b = np.asarray(ln2_b, np.float32)
    c_attn_w = np.asarray(c_attn_w, np.float32)
    c_attn_b = np.asarray(c_attn_b, np.float32)
    c_proj_w = np.asarray(c_proj_w, np.float32)
    c_proj_b = np.asarray(c_proj_b, np.float32)
    up_w = np.asarray(up_w, np.float32); up_b = np.asarray(up_b, np.float32)
    down_w = np.asarray(down_w, np.float32)
    down_b = np.asarray(down_b, np.float32)

    Wa = c_attn_w * ln1_w[None, :]
    ba = c_attn_b + c_attn_w @ ln1_b
    Wq, Wk, Wv = Wa[:C], Wa[C:2 * C], Wa[2 * C:]
    bqv, bkv, bvv = ba[:C], ba[C:2 * C], ba[2 * C:]
    s = 1.0 / np.sqrt(D)
    Wq = Wq * s; bqv = bqv * s

    Wup = up_w * ln2_w[None, :]
    bupv = up_b + up_w @ ln2_b

    def b2t(v, n):   # per-partition bias layout [128, n]
        return np.ascontiguousarray(v.reshape(n, P).T.astype(np.float32))

    # diag-tile masks [128 kv, 256 q] tiled x4 heads
    tk = np.arange(P)[:, None]
    tq = np.arange(P)[None, :]
    mA = (tk <= tq).astype(np.float32)
    blockA = np.concatenate([mA, np.ones((P, P), np.float32)], axis=1)
    blockB = np.concatenate([np.zeros((P, P), np.float32), mA], axis=1)
    maskA = np.tile(blockA, (1, 4)).astype(BF)
    maskB = np.tile(blockB, (1, 4)).astype(BF)

    shared = {
        "WupT": np.ascontiguousarray(Wup.T).astype(BF),
        "WdownT": np.ascontiguousarray(down_w.T).astype(BF),
        "bup": b2t(bupv, 32), "bdown": b2t(down_b, 8),
        "maskA": maskA, "maskB": maskB,
    }

    xb = x + c_proj_b[None, None, :]
    in_maps = []
    for core in range(8):
        b, j = core // 4, core % 4
        hsl = slice(256 * j, 256 * j + 256)
        m = dict(shared)
        m["gT"] = np.ascontiguousarray(g[b].T).astype(BF)
        m["WqT"] = np.ascontiguousarray(Wq[hsl].T).astype(BF)
        m["WkT"] = np.ascontiguousarray(Wk[hsl].T).astype(BF)
        m["WvT"] = np.ascontiguousarray(Wv[hsl].T).astype(BF)
        m["WpT"] = np.ascontiguousarray(c_proj_w[:, hsl].T).astype(BF)
        m["bq"] = b2t(bqv[hsl], 2)
        m["bk"] = b2t(bkv[hsl], 2)
        m["brep"] = np.broadcast_to(bvv[hsl].astype(BF), (P, 256)).copy()
        cols = np.r_[256 * j:256 * j + 256, 1024 + 256 * j:1024 + 256 * j + 256]
        m["xbT"] = np.ascontiguousarray(xb[b].T[:, cols]).astype(BF)
        in_maps.append(m)
    return in_maps


def kernel(**inputs):
    global _CACHED_NC
    if _CACHED_NC is None:
        _CACHED_NC = _build_nc()
    nc = _CACHED_NC
    in_maps = _prep_inputs(**inputs)
    try:
        res = run_bass_kernel_spmd(nc, in_maps, list(range(8)))
    except Exception:
        res = run_bass_kernel_spmd(nc, in_maps, list(range(8)))
    out = np.empty((B, T, C), np.float32)
    for core in range(8):
        o = res.results[core]["OUT"]                # [C, 512]
        b, j = core // 4, core % 4
        out[b, 256 * j:256 * j + 256, :] = o[:, 0:256].T
        out[b, 1024 + 256 * j:1024 + 256 * j + 256, :] = o[:, 256:512].T
    return out
